# revision 14
# baseline (speedup 1.0000x reference)
"""SEIR Euler integration kernel for 8 TRN2 NeuronCores — v2.

Shards the batch axis (B=32768) across 8 cores (4096 each); every core runs
the full 1024-step Euler scan on its shard and streams the trajectory to DRAM.

v2 reformulation (4 ops/step instead of 6), with all state scaled by
c1=beta/2 so the bilinear term becomes a PURE product (Pool has no
scalar_tensor_tensor; it can only run tensor_tensor / tensor_scalar):
  Scaled state: s = c1*c2*S, e = c1*c2*E, i = c1*I, r = -(c1/c3)*R
                (c1=beta/2, c2=sigma/2, c3=gamma/2)
  Per step:  D   = s * i                             [tt mult, Pool]
             s'  = s - D   &  r' = r - i    fused ONE tt sub [Pool]
             e'  = (a2*e) + D               [stt, DVE]  (a2=1-c2)
             i'  = (a3*i) + e               [stt, DVE]  (a3=1-c3)
  Host recovers S=s/(c1c2), E=e/(c1c2), I=i/c1, R=-r*c3/c1 (free rescale).

Per-core SBUF staging layout (c-outer): one fp32 tile [128, 5*K*32] per
K-step block, column = c*(K*32) + k*32 + g with slots c=0:D 1:s 2:r 3:e 4:i,
batch element b = p*32 + g.  Slots 1..4 of the whole block form ONE
contiguous 8KB-per-partition run -> single full-bandwidth DMA per block into
DRAM out[128, NBLK, 4*K*32]; host unscrambles.  The D slot is never DMA'd.

Toolchain constraint: this container's walrus build rejects instructions
carrying >2 semaphore waits.  We use two compute engines (DVE + POOL) plus
one HWDGE DMA sem lane, and legalize the emitted sync (see _build tail).
"""

import sys

sys.path.insert(0, "/opt/trn_rl_repo")

import numpy as np

import concourse.bass as bass
import concourse.tile as tile
import concourse.tile_sem_assignment as _tsa
from concourse import mybir
from concourse.bass_utils import run_bass_kernel_spmd

_tsa.NUM_HWDGE_SEMS = 1
_tsa.NUM_SWDGE_GLOBAL_SEMS = 1

T = 1024
B = 32768
NCORES = 8
BS = B // NCORES  # 4096 batch elements per core
P = 128  # SBUF partitions
G = BS // P  # 32 batch elements per partition
C = 5  # slots: D, s, r, e, i
K = 16  # steps per DMA block
NBLK = T // K
SEG = K * G  # columns per slot per block

TRACE = False
# which engine computes the e/i updates: "pool" (GPSIMD) or "dve"
EI_ENGINE = "pool"

f32 = mybir.dt.float32
mult = mybir.AluOpType.mult
add = mybir.AluOpType.add
subtract = mybir.AluOpType.subtract


def _build(t_total=T, ei_engine=EI_ENGINE, chain=False):
    nblk = t_total // K
    nc = bass.Bass(trn_type="TRN2")
    init = nc.dram_tensor("initial", [4, BS], f32, kind="ExternalInput")
    beta = nc.dram_tensor("beta", [1], f32, kind="ExternalInput")
    gamma = nc.dram_tensor("gamma", [1], f32, kind="ExternalInput")
    sigma = nc.dram_tensor("sigma", [1], f32, kind="ExternalInput")
    out = nc.dram_tensor("out", [P, nblk, 4 * SEG], f32, kind="ExternalOutput")
    chain_in = chain_out = None
    if chain:
        chain_in = nc.dram_tensor("chain", [1, 1], f32, kind="ExternalInput")
        chain_out = nc.dram_tensor("chain_out", [1, 1], f32, kind="ExternalOutput")

    def ei(name):
        eng = nc.gpsimd if ei_engine == "pool" else nc.vector
        return getattr(eng, name)

    with tile.TileContext(nc) as tc:
        with (
            tc.tile_pool(name="consts", bufs=1) as consts,
            tc.tile_pool(name="stage", bufs=3) as stagep,
        ):
            # ---- broadcast the three rate scalars to all partitions ----
            bt = consts.tile([P, 1], f32, tag="bt")
            gt = consts.tile([P, 1], f32, tag="gt")
            st = consts.tile([P, 1], f32, tag="st")
            for dst, src in ((bt, beta), (gt, gamma), (st, sigma)):
                src_ap = src[:]
                bcast = bass.AP(
                    tensor=src_ap.tensor,
                    offset=src_ap.offset,
                    ap=[[0, P], [1, 1]],
                )
                nc.sync.dma_start(out=dst[:, :], in_=bcast)

            # derived per-partition scalars (all prepared on DVE)
            k1t = consts.tile([P, 1], f32, tag="k1")  # c1 = beta/2
            a2t = consts.tile([P, 1], f32, tag="a2")  # 1 - sigma/2
            a3t = consts.tile([P, 1], f32, tag="a3")  # 1 - gamma/2
            c2t = consts.tile([P, 1], f32, tag="c2")  # sigma/2
            rg = consts.tile([P, 1], f32, tag="rg")  # 1/gamma
            nr3 = consts.tile([P, 1], f32, tag="nr3")  # -2/gamma
            m1t = consts.tile([P, 1], f32, tag="m1")  # c1*c2
            m2t = consts.tile([P, 1], f32, tag="m2")  # -c1*2/gamma
            nc.vector.tensor_scalar_mul(k1t[:, :], bt[:, :], 0.5)
            nc.vector.tensor_scalar(a2t[:, :], st[:, :], -0.5, 1.0, mult, add)
            nc.vector.tensor_scalar(a3t[:, :], gt[:, :], -0.5, 1.0, mult, add)
            nc.vector.tensor_scalar_mul(c2t[:, :], st[:, :], 0.5)
            nc.vector.reciprocal(rg[:, :], gt[:, :])
            nc.vector.tensor_scalar_mul(nr3[:, :], rg[:, :], -2.0)
            nc.vector.tensor_mul(m1t[:, :], k1t[:, :], c2t[:, :])
            nc.vector.tensor_mul(m2t[:, :], k1t[:, :], nr3[:, :])
            a2 = a2t[:, 0:1]
            a3 = a3t[:, 0:1]

            # ---- initial state into block 0, step row 0 ----
            tmp0 = consts.tile([P, 4 * G], f32, tag="init_tmp")
            nc.sync.dma_start(
                out=tmp0[:, :].rearrange("p (c g) -> p c g", c=4),
                in_=init[:, :].rearrange("c (p g) -> p c g", p=P),
            )
            tv = tmp0[:, :].rearrange("p (c g) -> p c g", c=4)  # S,E,I,R

            cur = stagep.tile([P, C * SEG], f32, tag="stage")
            r = cur[:, :].rearrange("p (c k g) -> p c k g", c=C, k=K)
            # row 0: s = c1c2*S ; r = -(c1/c3)*R ; e = c1c2*E ; i = c1*I
            # s,r slots are Pool-owned in the loop, e,i slots DVE-owned; the
            # init writes match so each block-store DMA has a single-engine
            # data dependency.  Pool allows ONE sync wait per instruction, so
            # it copies first (waits the input DMA) then scales in place
            # (waits DVE for the derived scalars).
            ei("tensor_copy")(r[:, 1, 0, :], tv[:, 0, :])
            ei("tensor_copy")(r[:, 2, 0, :], tv[:, 3, :])
            ei("tensor_scalar_mul")(r[:, 1, 0, :], r[:, 1, 0, :], m1t[:, 0:1])
            ei("tensor_scalar_mul")(r[:, 2, 0, :], r[:, 2, 0, :], m2t[:, 0:1])
            nc.vector.tensor_scalar_mul(r[:, 3, 0, :], tv[:, 1, :], m1t[:, 0:1])
            nc.vector.tensor_scalar_mul(r[:, 4, 0, :], tv[:, 2, :], k1t[:, 0:1])

            def sr_pair(rr, k):
                # [s_k | r_k] as a 2-run strided AP (outer stride = SEG)
                return rr[:, 1:3, k, :]

            def di_pair(rr, k):
                # [D_k | i_k] as a 2-run strided AP (outer stride = 4*SEG)
                return rr[:, 0:5:4, k, :]

            prev_r, prev_k = r, 0
            first = True
            for blk in range(nblk):
                if not first:
                    cur = stagep.tile([P, C * SEG], f32, tag="stage")
                    r = cur[:, :].rearrange("p (c k g) -> p c k g", c=C, k=K)
                ks = range(1, K) if first else range(K)
                first = False
                for k in ks:
                    pS = prev_r[:, 1, prev_k, :]
                    pE = prev_r[:, 3, prev_k, :]
                    pI = prev_r[:, 4, prev_k, :]
                    pD = prev_r[:, 0, prev_k, :]
                    # i' = (a3*i) + e   [DVE, no cross dep -> overlaps Pool.D]
                    nc.vector.scalar_tensor_tensor(
                        r[:, 4, k, :], pI, a3, pE, mult, add
                    )
                    # D = s*i  -> written into the PREVIOUS row's D slot
                    ei("tensor_mul")(pD, pS, pI)
                    # e' = (a2*e) + D   [DVE, waits Pool.D]
                    nc.vector.scalar_tensor_tensor(
                        r[:, 3, k, :], pE, a2, pD, mult, add
                    )
                    # [s' | r'] = [s | r] - [D | i]   [Pool]
                    ei("tensor_tensor")(
                        sr_pair(r, k),
                        sr_pair(prev_r, prev_k),
                        di_pair(prev_r, prev_k),
                        subtract,
                    )
                    prev_r, prev_k = r, k
                # store the block in two DMAs so each carries ONE data wait
                # (this walrus build allows a single sync wait per DMA):
                # slots s,r written by Pool; slots e,i written by DVE
                nc.sync.dma_start(
                    out=out[:, blk, 0 : 2 * SEG], in_=cur[:, SEG : 3 * SEG]
                )
                nc.sync.dma_start(
                    out=out[:, blk, 2 * SEG : 4 * SEG],
                    in_=cur[:, 3 * SEG : 5 * SEG],
                )

            if chain:
                cht = consts.tile([1, 1], f32, tag="chain")
                nc.sync.dma_start(out=cht[:, :], in_=chain_in[:, :])
                chv = consts.tile([1, 1], f32, tag="chainv")
                last_elem = r[0:1, 4, K - 1, 0:1]
                nc.vector.tensor_scalar_mul(chv[:, :], last_elem, cht[0:1, 0:1])
                nc.sync.dma_start(out=chain_out[:, :], in_=chv[:, :])

    _legalize_sync(nc)
    return nc


def _legalize_sync(nc):
    # Legalize for walrus' sync-wait limits (2 per compute instruction, 1 per
    # DMA/Pool instruction; see kernel_v1_baseline.py for the rationale):
    #  - DMACopy: drop DMA-lane ordering waits (FIFO ring + fixed +16 incs
    #    make them redundant when a data wait is present).
    #  - Drain: keep only the last DMA wait (the final block-store DMA waits
    #    on both compute engines, so DMA completion implies all engines).
    #  - Other instructions: drop sem-ge waits on their OWN engine's sem.
    for bb in nc.m.functions[0].blocks:
        for ins in bb.instructions:
            si = ins.sync_info
            if si is None:
                continue
            ow = si.on_wait
            if not ow or len(ow) < 2:
                continue
            kind = ins.__class__.__name__
            eng = str(ins.engine).rsplit(".", 1)[-1]
            if kind == "InstDMACopy":
                new_w = [
                    w
                    for w in ow
                    if not (
                        w.ant_name.startswith("DMAHW")
                        or w.ant_name.startswith("DMASW")
                    )
                ]
            elif kind == "InstDrain":
                dma_w = [w for w in ow if w.ant_name.startswith("DMA")]
                new_w = dma_w[-1:] if dma_w else ow[-1:]
            else:
                new_w = [
                    w
                    for w in ow
                    if not (
                        w.wait_mode == "sem-ge-imm"
                        and w.ant_name.split("_")[0] == eng
                    )
                ]
            if len(new_w) < len(ow):
                si.on_wait = new_w
                ins.sync_info = si


_nc = None


def kernel(initial, beta, gamma, sigma, t):
    global _nc
    assert int(t) == T
    initial = np.ascontiguousarray(np.asarray(initial, dtype=np.float32))
    beta = np.asarray(beta, dtype=np.float32).reshape(1)
    gamma = np.asarray(gamma, dtype=np.float32).reshape(1)
    sigma = np.asarray(sigma, dtype=np.float32).reshape(1)
    assert initial.shape == (4, B)

    if _nc is None:
        _nc = _build()

    in_maps = []
    for i in range(NCORES):
        shard = np.ascontiguousarray(initial[:, i * BS : (i + 1) * BS])
        in_maps.append(
            {"initial": shard, "beta": beta, "gamma": gamma, "sigma": sigma}
        )

    res = run_bass_kernel_spmd(
        _nc, in_maps, core_ids=list(range(NCORES)), trace=TRACE
    )
    if TRACE and res.exec_time_ns is not None:
        print(f"HW exec time: {res.exec_time_ns} ns")

    # host-side unscramble + unscale
    c1 = np.float64(beta[0]) / 2.0
    c2 = np.float64(sigma[0]) / 2.0
    c3 = np.float64(gamma[0]) / 2.0
    inv_m1 = np.float32(1.0 / (c1 * c2))
    inv_k1 = np.float32(1.0 / c1)
    neg_m2 = np.float32(-c3 / c1)

    full = np.empty((T, B, 4), dtype=np.float32)
    for i in range(NCORES):
        # out [P, NBLK, 4*K*G]: slot-major ((s,r,e,i), k, g) per block
        arr = res.results[i]["out"].reshape(P, NBLK, 4, K, G)
        # -> (T=blk*K+k, b_local=p*G+g, slot)
        arr = arr.transpose(1, 3, 0, 4, 2).reshape(T, BS, 4)
        dst = full[:, i * BS : (i + 1) * BS, :]
        dst[:, :, 0] = arr[:, :, 0] * inv_m1  # S = s/(c1*c2)
        dst[:, :, 1] = arr[:, :, 2] * inv_m1  # E = e/(c1*c2)
        dst[:, :, 2] = arr[:, :, 3] * inv_k1  # I = i/c1
        dst[:, :, 3] = arr[:, :, 1] * neg_m2  # R = -r*c3/c1
    return full.reshape(T * B, 4)


if __name__ == "__main__":
    rng = np.random.default_rng(0)
    ini = rng.random((4, B), dtype=np.float32)
    be, ga, si = (rng.random(1, dtype=np.float32) for _ in range(3))
    outv = kernel(ini, be, ga, si, T)
    print("ran, out shape", outv.shape, outv[:4])


# revision 20
# speedup vs baseline: 25.6714x; 25.6714x over previous
"""SEIR Euler integration kernel for 8 TRN2 NeuronCores — v2.

Shards the batch axis (B=32768) across 8 cores (4096 each); every core runs
the full 1024-step Euler scan on its shard and streams the trajectory to DRAM.

v2 reformulation (4 ops/step instead of 6), with all state scaled by
c1=beta/2 so the bilinear term becomes a PURE product (Pool has no
scalar_tensor_tensor; it can only run tensor_tensor / tensor_scalar):
  Scaled state: s = c1*c2*S, e = c1*c2*E, i = c1*I, r = -(c1/c3)*R
                (c1=beta/2, c2=sigma/2, c3=gamma/2)
  Per step:  D   = s * i                             [tt mult, Pool]
             s'  = s - D   &  r' = r - i    fused ONE tt sub [Pool]
             e'  = (a2*e) + D               [stt, DVE]  (a2=1-c2)
             i'  = (a3*i) + e               [stt, DVE]  (a3=1-c3)
  Host recovers S=s/(c1c2), E=e/(c1c2), I=i/c1, R=-r*c3/c1 (free rescale).

Per-core SBUF staging layout (c-outer): one fp32 tile [128, 5*K*32] per
K-step block, column = c*(K*32) + k*32 + g with slots c=0:D 1:s 2:r 3:e 4:i,
batch element b = p*32 + g.  Slots 1..4 of the whole block form ONE
contiguous 8KB-per-partition run -> single full-bandwidth DMA per block into
DRAM out[128, NBLK, 4*K*32]; host unscrambles.  The D slot is never DMA'd.

Toolchain constraint: this container's walrus build rejects instructions
carrying >2 semaphore waits.  We use two compute engines (DVE + POOL) plus
one HWDGE DMA sem lane, and legalize the emitted sync (see _build tail).
"""

import sys

sys.path.insert(0, "/opt/trn_rl_repo")

import numpy as np

import concourse.bass as bass
import concourse.tile as tile
import concourse.tile_sem_assignment as _tsa
from concourse import mybir
from concourse.bass_utils import run_bass_kernel_spmd

_tsa.NUM_HWDGE_SEMS = 1
_tsa.NUM_SWDGE_GLOBAL_SEMS = 1

T = 1024
B = 32768
NCORES = 8
BS = B // NCORES  # 4096 batch elements per core
P = 128  # SBUF partitions
G = BS // P  # 32 batch elements per partition
C = 5  # slots: D, s, r, e, i
K = 16  # steps per DMA block
NBLK = T // K
SEG = K * G  # columns per slot per block

TRACE = False
# which engine computes the e/i updates: "pool" (GPSIMD) or "dve"
EI_ENGINE = "pool"

f32 = mybir.dt.float32
mult = mybir.AluOpType.mult
add = mybir.AluOpType.add
subtract = mybir.AluOpType.subtract


def _build(t_total=T, ei_engine=EI_ENGINE, chain=False, passes=1, tiny_out=False):
    # passes>1 re-runs the whole integration (state carried over, same out
    # rows rewritten) inside one NEFF: per-call I/O is identical, so the
    # passes-differential isolates the true per-pass device time.
    # tiny_out=True (timing only) writes every block to out row 0 so the
    # per-call PJRT output copy shrinks from 64MB to 1MB per core.
    nblk = t_total // K
    out_blks = 1 if tiny_out else NBLK
    nc = bass.Bass(trn_type="TRN2")
    init = nc.dram_tensor("initial", [4, BS], f32, kind="ExternalInput")
    beta = nc.dram_tensor("beta", [1], f32, kind="ExternalInput")
    gamma = nc.dram_tensor("gamma", [1], f32, kind="ExternalInput")
    sigma = nc.dram_tensor("sigma", [1], f32, kind="ExternalInput")
    out = nc.dram_tensor(
        "out", [P, out_blks, 4 * SEG], f32, kind="ExternalOutput"
    )
    chain_in = chain_out = None
    if chain:
        chain_in = nc.dram_tensor("chain", [1, 1], f32, kind="ExternalInput")
        chain_out = nc.dram_tensor("chain_out", [1, 1], f32, kind="ExternalOutput")

    def ei(name):
        eng = nc.gpsimd if ei_engine == "pool" else nc.vector
        return getattr(eng, name)

    with tile.TileContext(nc) as tc:
        with (
            tc.tile_pool(name="consts", bufs=1) as consts,
            tc.tile_pool(name="stage", bufs=3) as stagep,
        ):
            # ---- broadcast the three rate scalars to all partitions ----
            bt = consts.tile([P, 1], f32, tag="bt")
            gt = consts.tile([P, 1], f32, tag="gt")
            st = consts.tile([P, 1], f32, tag="st")
            for dst, src in ((bt, beta), (gt, gamma), (st, sigma)):
                src_ap = src[:]
                bcast = bass.AP(
                    tensor=src_ap.tensor,
                    offset=src_ap.offset,
                    ap=[[0, P], [1, 1]],
                )
                nc.sync.dma_start(out=dst[:, :], in_=bcast)

            # derived per-partition scalars (all prepared on DVE)
            k1t = consts.tile([P, 1], f32, tag="k1")  # c1 = beta/2
            a2t = consts.tile([P, 1], f32, tag="a2")  # 1 - sigma/2
            a3t = consts.tile([P, 1], f32, tag="a3")  # 1 - gamma/2
            c2t = consts.tile([P, 1], f32, tag="c2")  # sigma/2
            rg = consts.tile([P, 1], f32, tag="rg")  # 1/gamma
            nr3 = consts.tile([P, 1], f32, tag="nr3")  # -2/gamma
            m1t = consts.tile([P, 1], f32, tag="m1")  # c1*c2
            m2t = consts.tile([P, 1], f32, tag="m2")  # -c1*2/gamma
            nc.vector.tensor_scalar_mul(k1t[:, :], bt[:, :], 0.5)
            nc.vector.tensor_scalar(a2t[:, :], st[:, :], -0.5, 1.0, mult, add)
            nc.vector.tensor_scalar(a3t[:, :], gt[:, :], -0.5, 1.0, mult, add)
            nc.vector.tensor_scalar_mul(c2t[:, :], st[:, :], 0.5)
            nc.vector.reciprocal(rg[:, :], gt[:, :])
            nc.vector.tensor_scalar_mul(nr3[:, :], rg[:, :], -2.0)
            nc.vector.tensor_mul(m1t[:, :], k1t[:, :], c2t[:, :])
            nc.vector.tensor_mul(m2t[:, :], k1t[:, :], nr3[:, :])
            a2 = a2t[:, 0:1]
            a3 = a3t[:, 0:1]

            # ---- initial state into block 0, step row 0 ----
            tmp0 = consts.tile([P, 4 * G], f32, tag="init_tmp")
            nc.sync.dma_start(
                out=tmp0[:, :].rearrange("p (c g) -> p c g", c=4),
                in_=init[:, :].rearrange("c (p g) -> p c g", p=P),
            )
            tv = tmp0[:, :].rearrange("p (c g) -> p c g", c=4)  # S,E,I,R

            cur = stagep.tile([P, C * SEG], f32, tag="stage")
            r = cur[:, :].rearrange("p (c k g) -> p c k g", c=C, k=K)
            # row 0: s = c1c2*S ; r = -(c1/c3)*R ; e = c1c2*E ; i = c1*I
            # s,r slots are Pool-owned in the loop, e,i slots DVE-owned; the
            # init writes match so each block-store DMA has a single-engine
            # data dependency.  Pool allows ONE sync wait per instruction, so
            # it copies first (waits the input DMA) then scales in place
            # (waits DVE for the derived scalars).
            ei("tensor_copy")(r[:, 1, 0, :], tv[:, 0, :])
            ei("tensor_copy")(r[:, 2, 0, :], tv[:, 3, :])
            ei("tensor_scalar_mul")(r[:, 1, 0, :], r[:, 1, 0, :], m1t[:, 0:1])
            ei("tensor_scalar_mul")(r[:, 2, 0, :], r[:, 2, 0, :], m2t[:, 0:1])
            nc.vector.tensor_scalar_mul(r[:, 3, 0, :], tv[:, 1, :], m1t[:, 0:1])
            nc.vector.tensor_scalar_mul(r[:, 4, 0, :], tv[:, 2, :], k1t[:, 0:1])

            def sr_pair(rr, k):
                # [s_k | r_k] as a 2-run strided AP (outer stride = SEG)
                return rr[:, 1:3, k, :]

            def di_pair(rr, k):
                # [D_k | i_k] as a 2-run strided AP (outer stride = 4*SEG)
                return rr[:, 0:5:4, k, :]

            prev_r, prev_k = r, 0
            first = True
            for blk in range(nblk * passes):
                blk_out = (blk % nblk) % out_blks
                if not first:
                    cur = stagep.tile([P, C * SEG], f32, tag="stage")
                    r = cur[:, :].rearrange("p (c k g) -> p c k g", c=C, k=K)
                ks = range(1, K) if first else range(K)
                first = False
                for k in ks:
                    pS = prev_r[:, 1, prev_k, :]
                    pE = prev_r[:, 3, prev_k, :]
                    pI = prev_r[:, 4, prev_k, :]
                    pD = prev_r[:, 0, prev_k, :]
                    # i' = (a3*i) + e   [DVE, no cross dep -> overlaps Pool.D]
                    nc.vector.scalar_tensor_tensor(
                        r[:, 4, k, :], pI, a3, pE, mult, add
                    )
                    # D = s*i  -> written into the PREVIOUS row's D slot
                    ei("tensor_mul")(pD, pS, pI)
                    # e' = (a2*e) + D   [DVE, waits Pool.D]
                    nc.vector.scalar_tensor_tensor(
                        r[:, 3, k, :], pE, a2, pD, mult, add
                    )
                    # [s' | r'] = [s | r] - [D | i]   [Pool]
                    ei("tensor_tensor")(
                        sr_pair(r, k),
                        sr_pair(prev_r, prev_k),
                        di_pair(prev_r, prev_k),
                        subtract,
                    )
                    prev_r, prev_k = r, k
                # store the block in two DMAs so each carries ONE data wait
                # (this walrus build allows a single sync wait per DMA):
                # slots s,r written by Pool; slots e,i written by DVE
                nc.sync.dma_start(
                    out=out[:, blk_out, 0 : 2 * SEG], in_=cur[:, SEG : 3 * SEG]
                )
                nc.sync.dma_start(
                    out=out[:, blk_out, 2 * SEG : 4 * SEG],
                    in_=cur[:, 3 * SEG : 5 * SEG],
                )

            if chain:
                cht = consts.tile([1, 1], f32, tag="chain")
                nc.sync.dma_start(out=cht[:, :], in_=chain_in[:, :])
                chv = consts.tile([1, 1], f32, tag="chainv")
                last_elem = r[0:1, 4, K - 1, 0:1]
                nc.vector.tensor_scalar_mul(chv[:, :], last_elem, cht[0:1, 0:1])
                nc.sync.dma_start(out=chain_out[:, :], in_=chv[:, :])

    _legalize_sync(nc)
    return nc


def _legalize_sync(nc):
    # Legalize for walrus' sync-wait limits (2 per compute instruction, 1 per
    # DMA/Pool instruction; see kernel_v1_baseline.py for the rationale):
    #  - DMACopy: drop DMA-lane ordering waits (FIFO ring + fixed +16 incs
    #    make them redundant when a data wait is present).
    #  - Drain: keep only the last DMA wait (the final block-store DMA waits
    #    on both compute engines, so DMA completion implies all engines).
    #  - Other instructions: drop sem-ge waits on their OWN engine's sem.
    for bb in nc.m.functions[0].blocks:
        for ins in bb.instructions:
            si = ins.sync_info
            if si is None:
                continue
            ow = si.on_wait
            if not ow or len(ow) < 2:
                continue
            kind = ins.__class__.__name__
            eng = str(ins.engine).rsplit(".", 1)[-1]
            if kind == "InstDMACopy":
                new_w = [
                    w
                    for w in ow
                    if not (
                        w.ant_name.startswith("DMAHW")
                        or w.ant_name.startswith("DMASW")
                    )
                ]
            elif kind == "InstDrain":
                dma_w = [w for w in ow if w.ant_name.startswith("DMA")]
                new_w = dma_w[-1:] if dma_w else ow[-1:]
            else:
                new_w = [
                    w
                    for w in ow
                    if not (
                        w.wait_mode == "sem-ge-imm"
                        and w.ant_name.split("_")[0] == eng
                    )
                ]
            if len(new_w) < len(ow):
                si.on_wait = new_w
                ins.sync_info = si


_nc = None


def kernel(initial, beta, gamma, sigma, t):
    global _nc
    assert int(t) == T
    initial = np.ascontiguousarray(np.asarray(initial, dtype=np.float32))
    beta = np.asarray(beta, dtype=np.float32).reshape(1)
    gamma = np.asarray(gamma, dtype=np.float32).reshape(1)
    sigma = np.asarray(sigma, dtype=np.float32).reshape(1)
    assert initial.shape == (4, B)

    if _nc is None:
        _nc = _build()

    in_maps = []
    for i in range(NCORES):
        shard = np.ascontiguousarray(initial[:, i * BS : (i + 1) * BS])
        in_maps.append(
            {"initial": shard, "beta": beta, "gamma": gamma, "sigma": sigma}
        )

    res = run_bass_kernel_spmd(
        _nc, in_maps, core_ids=list(range(NCORES)), trace=TRACE
    )
    if TRACE and res.exec_time_ns is not None:
        print(f"HW exec time: {res.exec_time_ns} ns")

    # host-side unscramble + unscale
    c1 = np.float64(beta[0]) / 2.0
    c2 = np.float64(sigma[0]) / 2.0
    c3 = np.float64(gamma[0]) / 2.0
    inv_m1 = np.float32(1.0 / (c1 * c2))
    inv_k1 = np.float32(1.0 / c1)
    neg_m2 = np.float32(-c3 / c1)

    full = np.empty((T, B, 4), dtype=np.float32)
    for i in range(NCORES):
        # out [P, NBLK, 4*K*G]: slot-major ((s,r,e,i), k, g) per block
        arr = res.results[i]["out"].reshape(P, NBLK, 4, K, G)
        # -> (T=blk*K+k, b_local=p*G+g, slot)
        arr = arr.transpose(1, 3, 0, 4, 2).reshape(T, BS, 4)
        dst = full[:, i * BS : (i + 1) * BS, :]
        dst[:, :, 0] = arr[:, :, 0] * inv_m1  # S = s/(c1*c2)
        dst[:, :, 1] = arr[:, :, 2] * inv_m1  # E = e/(c1*c2)
        dst[:, :, 2] = arr[:, :, 3] * inv_k1  # I = i/c1
        dst[:, :, 3] = arr[:, :, 1] * neg_m2  # R = -r*c3/c1
    return full.reshape(T * B, 4)


if __name__ == "__main__":
    rng = np.random.default_rng(0)
    ini = rng.random((4, B), dtype=np.float32)
    be, ga, si = (rng.random(1, dtype=np.float32) for _ in range(3))
    outv = kernel(ini, be, ga, si, T)
    print("ran, out shape", outv.shape, outv[:4])


# revision 24
# speedup vs baseline: 55.8147x; 2.1742x over previous
"""SEIR Euler integration kernel for 8 TRN2 NeuronCores — v2.

Shards the batch axis (B=32768) across 8 cores (4096 each); every core runs
the full 1024-step Euler scan on its shard and streams the trajectory to DRAM.

v2 reformulation (4 ops/step instead of 6), with all state scaled by
c1=beta/2 so the bilinear term becomes a PURE product (Pool has no
scalar_tensor_tensor; it can only run tensor_tensor / tensor_scalar):
  Scaled state: s = c1*c2*S, e = c1*c2*E, i = c1*I, r = -(c1/c3)*R
                (c1=beta/2, c2=sigma/2, c3=gamma/2)
  Per step:  D   = s * i                             [tt mult, Pool]
             s'  = s - D   &  r' = r - i    fused ONE tt sub [Pool]
             e'  = (a2*e) + D               [stt, DVE]  (a2=1-c2)
             i'  = (a3*i) + e               [stt, DVE]  (a3=1-c3)
  Host recovers S=s/(c1c2), E=e/(c1c2), I=i/c1, R=-r*c3/c1 (free rescale).

Per-core SBUF staging layout (c-outer): one fp32 tile [128, 5*K*32] per
K-step block, column = c*(K*32) + k*32 + g with slots c=0:D 1:s 2:r 3:e 4:i,
batch element b = p*32 + g.  Slots 1..4 of the whole block form ONE
contiguous 8KB-per-partition run -> single full-bandwidth DMA per block into
DRAM out[128, NBLK, 4*K*32]; host unscrambles.  The D slot is never DMA'd.

Toolchain constraint: this container's walrus build rejects instructions
carrying >2 semaphore waits.  We use two compute engines (DVE + POOL) plus
one HWDGE DMA sem lane, and legalize the emitted sync (see _build tail).
"""

import sys

sys.path.insert(0, "/opt/trn_rl_repo")

import numpy as np

import concourse.bass as bass
import concourse.tile as tile
import concourse.tile_sem_assignment as _tsa
from concourse import mybir
from concourse.bass_utils import run_bass_kernel_spmd

_tsa.NUM_HWDGE_SEMS = 1
_tsa.NUM_SWDGE_GLOBAL_SEMS = 1

T = 1024
B = 32768
NCORES = 8
BS = B // NCORES  # 4096 batch elements per core
P = 128  # SBUF partitions
G = BS // P  # 32 batch elements per partition
C = 5  # slots: D, s, r, e, i
K = 16  # steps per DMA block
NBLK = T // K
SEG = K * G  # columns per slot per block

TRACE = False
# which engine computes the e/i updates: "pool" (GPSIMD) or "dve"
EI_ENGINE = "dve"

f32 = mybir.dt.float32
mult = mybir.AluOpType.mult
add = mybir.AluOpType.add
subtract = mybir.AluOpType.subtract


def _build(t_total=T, ei_engine=EI_ENGINE, chain=False, passes=1, tiny_out=False,
           k_steps=K, bufs=3, d_op="tt", sr_split=False, scratch_out=False):
    # passes>1 re-runs the whole integration (state carried over, same out
    # rows rewritten) inside one NEFF: per-call I/O is identical, so the
    # passes-differential isolates the true per-pass device time.
    # tiny_out=True (timing only) writes every block to out row 0 so the
    # per-call PJRT output copy shrinks from 64MB to 1MB per core.
    kk = k_steps
    seg = kk * G
    nblk = t_total // kk
    out_blks = 1 if tiny_out else (T // kk)
    nc = bass.Bass(trn_type="TRN2")
    init = nc.dram_tensor("initial", [4, BS], f32, kind="ExternalInput")
    beta = nc.dram_tensor("beta", [1], f32, kind="ExternalInput")
    gamma = nc.dram_tensor("gamma", [1], f32, kind="ExternalInput")
    sigma = nc.dram_tensor("sigma", [1], f32, kind="ExternalInput")
    out = nc.dram_tensor(
        "out", [P, out_blks, 4 * seg], f32, kind="ExternalOutput"
    )
    if scratch_out:
        # timing mode: full-size Internal scratch gets the real block DMAs
        # (distinct HBM addresses), while the tiny ExternalOutput keeps the
        # per-call PJRT I/O at ~1MB.
        outd = nc.dram_tensor(
            "outd", [P, T // kk, 4 * seg], f32, kind="Internal"
        )
    chain_in = chain_out = None
    if chain:
        chain_in = nc.dram_tensor("chain", [1, 1], f32, kind="ExternalInput")
        chain_out = nc.dram_tensor("chain_out", [1, 1], f32, kind="ExternalOutput")

    def ei(name):
        eng = nc.gpsimd if ei_engine == "pool" else nc.vector
        return getattr(eng, name)

    with tile.TileContext(nc) as tc:
        with (
            tc.tile_pool(name="consts", bufs=1) as consts,
            tc.tile_pool(name="stage", bufs=bufs) as stagep,
        ):
            # ---- broadcast the three rate scalars to all partitions ----
            bt = consts.tile([P, 1], f32, tag="bt")
            gt = consts.tile([P, 1], f32, tag="gt")
            st = consts.tile([P, 1], f32, tag="st")
            for dst, src in ((bt, beta), (gt, gamma), (st, sigma)):
                src_ap = src[:]
                bcast = bass.AP(
                    tensor=src_ap.tensor,
                    offset=src_ap.offset,
                    ap=[[0, P], [1, 1]],
                )
                nc.sync.dma_start(out=dst[:, :], in_=bcast)

            # derived per-partition scalars (all prepared on DVE)
            k1t = consts.tile([P, 1], f32, tag="k1")  # c1 = beta/2
            a2t = consts.tile([P, 1], f32, tag="a2")  # 1 - sigma/2
            a3t = consts.tile([P, 1], f32, tag="a3")  # 1 - gamma/2
            c2t = consts.tile([P, 1], f32, tag="c2")  # sigma/2
            rg = consts.tile([P, 1], f32, tag="rg")  # 1/gamma
            nr3 = consts.tile([P, 1], f32, tag="nr3")  # -2/gamma
            m1t = consts.tile([P, 1], f32, tag="m1")  # c1*c2
            m2t = consts.tile([P, 1], f32, tag="m2")  # -c1*2/gamma
            nc.vector.tensor_scalar_mul(k1t[:, :], bt[:, :], 0.5)
            nc.vector.tensor_scalar(a2t[:, :], st[:, :], -0.5, 1.0, mult, add)
            nc.vector.tensor_scalar(a3t[:, :], gt[:, :], -0.5, 1.0, mult, add)
            nc.vector.tensor_scalar_mul(c2t[:, :], st[:, :], 0.5)
            nc.vector.reciprocal(rg[:, :], gt[:, :])
            nc.vector.tensor_scalar_mul(nr3[:, :], rg[:, :], -2.0)
            nc.vector.tensor_mul(m1t[:, :], k1t[:, :], c2t[:, :])
            nc.vector.tensor_mul(m2t[:, :], k1t[:, :], nr3[:, :])
            a2 = a2t[:, 0:1]
            a3 = a3t[:, 0:1]

            # ---- initial state into block 0, step row 0 ----
            tmp0 = consts.tile([P, 4 * G], f32, tag="init_tmp")
            nc.sync.dma_start(
                out=tmp0[:, :].rearrange("p (c g) -> p c g", c=4),
                in_=init[:, :].rearrange("c (p g) -> p c g", p=P),
            )
            tv = tmp0[:, :].rearrange("p (c g) -> p c g", c=4)  # S,E,I,R

            cur = stagep.tile([P, C * seg], f32, tag="stage")
            r = cur[:, :].rearrange("p (c k g) -> p c k g", c=C, k=kk)
            # row 0: s = c1c2*S ; r = -(c1/c3)*R ; e = c1c2*E ; i = c1*I
            # s,r slots are Pool-owned in the loop, e,i slots DVE-owned; the
            # init writes match so each block-store DMA has a single-engine
            # data dependency.  Pool allows ONE sync wait per instruction, so
            # it copies first (waits the input DMA) then scales in place
            # (waits DVE for the derived scalars).
            ei("tensor_copy")(r[:, 1, 0, :], tv[:, 0, :])
            ei("tensor_copy")(r[:, 2, 0, :], tv[:, 3, :])
            ei("tensor_scalar_mul")(r[:, 1, 0, :], r[:, 1, 0, :], m1t[:, 0:1])
            ei("tensor_scalar_mul")(r[:, 2, 0, :], r[:, 2, 0, :], m2t[:, 0:1])
            nc.vector.tensor_scalar_mul(r[:, 3, 0, :], tv[:, 1, :], m1t[:, 0:1])
            nc.vector.tensor_scalar_mul(r[:, 4, 0, :], tv[:, 2, :], k1t[:, 0:1])

            def sr_pair(rr, k):
                # [s_k | r_k] as a 2-run strided AP (outer stride = SEG)
                return rr[:, 1:3, k, :]

            def di_pair(rr, k):
                # [D_k | i_k] as a 2-run strided AP (outer stride = 4*SEG)
                return rr[:, 0:5:4, k, :]

            prev_r, prev_k = r, 0
            first = True
            for blk in range(nblk * passes):
                if scratch_out:
                    dma_dst, blk_out = outd, blk % nblk
                else:
                    dma_dst, blk_out = out, (blk % nblk) % out_blks
                if not first:
                    cur = stagep.tile([P, C * seg], f32, tag="stage")
                    r = cur[:, :].rearrange("p (c k g) -> p c k g", c=C, k=kk)
                ks = range(1, kk) if first else range(kk)
                first = False
                for k in ks:
                    pS = prev_r[:, 1, prev_k, :]
                    pE = prev_r[:, 3, prev_k, :]
                    pI = prev_r[:, 4, prev_k, :]
                    pD = prev_r[:, 0, prev_k, :]
                    # i' = (a3*i) + e   [DVE, no cross dep -> overlaps Pool.D]
                    nc.vector.scalar_tensor_tensor(
                        r[:, 4, k, :], pI, a3, pE, mult, add
                    )
                    # D = s*i  -> written into the PREVIOUS row's D slot
                    if d_op == "stt":
                        nc.vector.scalar_tensor_tensor(
                            pD, pS, 1.0, pI, mult, mult
                        )
                    else:
                        ei("tensor_mul")(pD, pS, pI)
                    # e' = (a2*e) + D   [DVE, waits Pool.D]
                    nc.vector.scalar_tensor_tensor(
                        r[:, 3, k, :], pE, a2, pD, mult, add
                    )
                    # [s' | r'] = [s | r] - [D | i]
                    if sr_split:
                        ei("tensor_tensor")(
                            r[:, 1, k, :], pS, pD, subtract
                        )
                        ei("tensor_tensor")(
                            r[:, 2, k, :], prev_r[:, 2, prev_k, :], pI, subtract
                        )
                    else:
                        ei("tensor_tensor")(
                            sr_pair(r, k),
                            sr_pair(prev_r, prev_k),
                            di_pair(prev_r, prev_k),
                            subtract,
                        )
                    prev_r, prev_k = r, k
                # store the block in two DMAs so each carries ONE data wait
                # (this walrus build allows a single sync wait per DMA):
                # slots s,r written by Pool; slots e,i written by DVE
                nc.sync.dma_start(
                    out=dma_dst[:, blk_out, 0 : 2 * seg],
                    in_=cur[:, seg : 3 * seg],
                )
                nc.sync.dma_start(
                    out=dma_dst[:, blk_out, 2 * seg : 4 * seg],
                    in_=cur[:, 3 * seg : 5 * seg],
                )

            if chain:
                cht = consts.tile([1, 1], f32, tag="chain")
                nc.sync.dma_start(out=cht[:, :], in_=chain_in[:, :])
                chv = consts.tile([1, 1], f32, tag="chainv")
                last_elem = r[0:1, 4, kk - 1, 0:1]
                nc.vector.tensor_scalar_mul(chv[:, :], last_elem, cht[0:1, 0:1])
                nc.sync.dma_start(out=chain_out[:, :], in_=chv[:, :])

    _legalize_sync(nc)
    return nc


def _legalize_sync(nc):
    # Legalize for walrus' sync-wait limits (2 per compute instruction, 1 per
    # DMA/Pool instruction; see kernel_v1_baseline.py for the rationale):
    #  - DMACopy: drop DMA-lane ordering waits (FIFO ring + fixed +16 incs
    #    make them redundant when a data wait is present).
    #  - Drain: keep only the last DMA wait (the final block-store DMA waits
    #    on both compute engines, so DMA completion implies all engines).
    #  - Other instructions: drop sem-ge waits on their OWN engine's sem.
    for bb in nc.m.functions[0].blocks:
        for ins in bb.instructions:
            si = ins.sync_info
            if si is None:
                continue
            ow = si.on_wait
            if not ow or len(ow) < 2:
                continue
            kind = ins.__class__.__name__
            eng = str(ins.engine).rsplit(".", 1)[-1]
            if kind == "InstDMACopy":
                new_w = [
                    w
                    for w in ow
                    if not (
                        w.ant_name.startswith("DMAHW")
                        or w.ant_name.startswith("DMASW")
                    )
                ]
            elif kind == "InstDrain":
                dma_w = [w for w in ow if w.ant_name.startswith("DMA")]
                new_w = dma_w[-1:] if dma_w else ow[-1:]
            else:
                new_w = [
                    w
                    for w in ow
                    if not (
                        w.wait_mode == "sem-ge-imm"
                        and w.ant_name.split("_")[0] == eng
                    )
                ]
            if len(new_w) < len(ow):
                si.on_wait = new_w
                ins.sync_info = si


_nc = None


def kernel(initial, beta, gamma, sigma, t):
    global _nc
    assert int(t) == T
    initial = np.ascontiguousarray(np.asarray(initial, dtype=np.float32))
    beta = np.asarray(beta, dtype=np.float32).reshape(1)
    gamma = np.asarray(gamma, dtype=np.float32).reshape(1)
    sigma = np.asarray(sigma, dtype=np.float32).reshape(1)
    assert initial.shape == (4, B)

    if _nc is None:
        _nc = _build()

    in_maps = []
    for i in range(NCORES):
        shard = np.ascontiguousarray(initial[:, i * BS : (i + 1) * BS])
        in_maps.append(
            {"initial": shard, "beta": beta, "gamma": gamma, "sigma": sigma}
        )

    res = run_bass_kernel_spmd(
        _nc, in_maps, core_ids=list(range(NCORES)), trace=TRACE
    )
    if TRACE and res.exec_time_ns is not None:
        print(f"HW exec time: {res.exec_time_ns} ns")

    # host-side unscramble + unscale
    c1 = np.float64(beta[0]) / 2.0
    c2 = np.float64(sigma[0]) / 2.0
    c3 = np.float64(gamma[0]) / 2.0
    inv_m1 = np.float32(1.0 / (c1 * c2))
    inv_k1 = np.float32(1.0 / c1)
    neg_m2 = np.float32(-c3 / c1)

    full = np.empty((T, B, 4), dtype=np.float32)
    for i in range(NCORES):
        # out [P, NBLK, 4*K*G]: slot-major ((s,r,e,i), k, g) per block
        arr = res.results[i]["out"].reshape(P, NBLK, 4, K, G)
        # -> (T=blk*K+k, b_local=p*G+g, slot)
        arr = arr.transpose(1, 3, 0, 4, 2).reshape(T, BS, 4)
        dst = full[:, i * BS : (i + 1) * BS, :]
        dst[:, :, 0] = arr[:, :, 0] * inv_m1  # S = s/(c1*c2)
        dst[:, :, 1] = arr[:, :, 2] * inv_m1  # E = e/(c1*c2)
        dst[:, :, 2] = arr[:, :, 3] * inv_k1  # I = i/c1
        dst[:, :, 3] = arr[:, :, 1] * neg_m2  # R = -r*c3/c1
    return full.reshape(T * B, 4)


if __name__ == "__main__":
    rng = np.random.default_rng(0)
    ini = rng.random((4, B), dtype=np.float32)
    be, ga, si = (rng.random(1, dtype=np.float32) for _ in range(3))
    outv = kernel(ini, be, ga, si, T)
    print("ran, out shape", outv.shape, outv[:4])
